# revision 6
# baseline (speedup 1.0000x reference)
"""Trainium2 Bass kernel for nn_MemoryEfficientAttnBlock (windowed attention block).

Reference computation (B=4, C=512, H=W=64, WS=32, NHEADS=8, GROUPS=32):
  h = GroupNorm(x) -> window partition (2x2 windows of 32x32) -> q,k,v 1x1 convs
  -> per-(window, head) softmax attention over n=1024 positions, d=64
  -> window reverse -> output 1x1 conv -> residual add.

Sharding: data-parallel across the 8 cores: core c handles batch c//2,
spatial half c%2 (rows hi*32..hi*32+31 = 2 windows of 32x32). Conv weights
replicated. GroupNorm statistics span the full batch, so each core also
reads the *other* half of its batch (stats only, no collectives).

Device-side design notes:
  - x and the conv weights are pre-cast to bf16 on the host, so the
    prologue DMA is ~6MB instead of ~10MB; fp32 x streams in lazily and is
    only read by the final residual add.
  - GroupNorm is applied explicitly: xn = A*x + B per channel with
    A = rstd*gamma, B = beta - mu*A (stats via bn_stats/bn_aggr and a tiny
    indicator matmul for the 16-channel groups). All GEMMs then run bf16.
  - Scores are computed directly transposed, S^T[m,n] = k^T q, so the
    softmax needs no max-pass (|s*scale| < ~2 here) and no transposes.
    Heads are processed in pairs 2j/2j+1: each has a K=64 contraction on
    disjoint partition halves of the same q/k tile, so with
    tile_position (0,0)/(64,0) both run concurrently in the PE array
    (2x row tiling).
  - exp(scores) and v^T are stored as fp8e4m3; attn@V contracts them on the
    PE at full rate. Each head's 128-col block of the vt tile is
    [v_h | ones], so one accumulation chain yields both the unnormalized
    output and the softmax row-sums on the other partition half.
  - Deferred work (next window's projections/vt, previous window's output
    projection) is interleaved into the exp-wait gaps inside the scores
    mc-loop, since the PE executes its queue in order.
"""

from collections import deque

import numpy as np

import concourse.bass as bass
import concourse.tile as tile
from concourse import bacc, mybir

f32 = mybir.dt.float32
bf16 = mybir.dt.bfloat16
fp8 = mybir.dt.float8e4
FT = mybir.ActivationFunctionType
OP = mybir.AluOpType

B, C, H, W = 4, 512, 64, 64
WS, NHEADS, D = 32, 8, 64
GROUPS, EPS = 32, 1e-6
SCALE = 1.0 / 8.0          # 1/sqrt(D)
NCH = C // 128             # 4 channel chunks
NWIN = 2                   # windows per core
N = WS * WS                # 1024 positions per window
NPOS = NWIN * N            # 2048 positions per core
NCORES = 8

ROW_TILE = True            # pack head pairs into the PE array for scores
ES_FP8 = True              # exp(scores)/v in fp8e4m3 for the attn@V GEMM


def build_kernel(reps: int = 1, loop_iters: int | None = None,
                 row_tile: bool = ROW_TILE, es_fp8: bool = ES_FP8):
    """Build + compile the per-core Bass program. Returns the Bacc object."""
    nc = bacc.Bacc("TRN2", target_bir_lowering=False, debug=False,
                   num_devices=NCORES)

    xb_d = nc.dram_tensor("xb", [C, NPOS], bf16, kind="ExternalInput").ap()
    xob_d = nc.dram_tensor("xob", [C, NPOS], bf16, kind="ExternalInput").ap()
    xm_d = nc.dram_tensor("xm", [C, NPOS], f32, kind="ExternalInput").ap()
    wq_d = nc.dram_tensor("wqT", [C, C], bf16, kind="ExternalInput").ap()
    wk_d = nc.dram_tensor("wkT", [C, C], bf16, kind="ExternalInput").ap()
    wv_d = nc.dram_tensor("wvT", [C, C], bf16, kind="ExternalInput").ap()
    wo_d = nc.dram_tensor("woT", [C, C], bf16, kind="ExternalInput").ap()
    gsc_d = nc.dram_tensor("gscale", [128, NCH], f32, kind="ExternalInput").ap()
    gbi_d = nc.dram_tensor("gbias", [128, NCH], f32, kind="ExternalInput").ap()
    bq_d = nc.dram_tensor("bq", [128, NCH], f32, kind="ExternalInput").ap()
    bk_d = nc.dram_tensor("bk", [128, NCH], f32, kind="ExternalInput").ap()
    bo_d = nc.dram_tensor("bo", [128, NCH], f32, kind="ExternalInput").ap()
    bv_d = nc.dram_tensor("bv", [1, C], f32, kind="ExternalInput").ap()
    g_d = nc.dram_tensor("G", [128, 8], f32, kind="ExternalInput").ap()
    gt_d = nc.dram_tensor("Gt", [8, 128], f32, kind="ExternalInput").ap()
    out_d = nc.dram_tensor("out", [C, NPOS], f32, kind="ExternalOutput").ap()

    es_dt = fp8 if es_fp8 else bf16

    with tile.TileContext(nc) as tc:
        with (
            tc.tile_pool(name="persist", bufs=1) as P,
            tc.tile_pool(name="stats", bufs=1) as ST,
            tc.tile_pool(name="xo", bufs=1) as XO,
            tc.tile_pool(name="qk", bufs=2) as QK,
            tc.tile_pool(name="vt", bufs=2) as VT,
            tc.tile_pool(name="es", bufs=2) as ES,
            tc.tile_pool(name="ao", bufs=2) as AO,
            tc.tile_pool(name="rr", bufs=1) as RR,
            tc.tile_pool(name="osb", bufs=2) as OS,
            tc.tile_pool(name="ps_proj", bufs=2, space="PSUM") as PSP,
            tc.tile_pool(name="ps_sc", bufs=2, space="PSUM") as PSS,
            tc.tile_pool(name="ps_av", bufs=2, space="PSUM") as PSA,
        ):
            # ---- persistent loads (once); critical-path first ----
            xb_sb = []
            for kc in range(NCH):
                t = P.tile([128, NPOS], bf16, tag=f"xb{kc}")
                nc.sync.dma_start(out=t, in_=xb_d[kc * 128:(kc + 1) * 128, :])
                xb_sb.append(t)
            xo_tiles = []
            for kc in range(NCH):
                for hh in range(2):
                    xo_t = XO.tile([128, N], bf16, tag=f"xo{2 * kc + hh}")
                    nc.sync.dma_start(
                        out=xo_t,
                        in_=xob_d[kc * 128:(kc + 1) * 128, hh * N:(hh + 1) * N])
                    xo_tiles.append(xo_t)
            wt = {}
            for nm, d in (("q", wq_d), ("k", wk_d), ("v", wv_d), ("o", wo_d)):
                wt[nm] = []
                for kc in range(NCH):
                    t = P.tile([128, C], bf16, tag=f"w{nm}{kc}")
                    nc.sync.dma_start(out=t, in_=d[kc * 128:(kc + 1) * 128, :])
                    wt[nm].append(t)
            gsc = P.tile([128, NCH], f32, tag="gsc")
            nc.sync.dma_start(out=gsc, in_=gsc_d)
            gbi = P.tile([128, NCH], f32, tag="gbi")
            nc.sync.dma_start(out=gbi, in_=gbi_d)
            bqc = P.tile([128, NCH], f32, tag="bqc")
            nc.sync.dma_start(out=bqc, in_=bq_d)
            bkc = P.tile([128, NCH], f32, tag="bkc")
            nc.sync.dma_start(out=bkc, in_=bk_d)
            boc = P.tile([128, NCH], f32, tag="boc")
            nc.sync.dma_start(out=boc, in_=bo_d)
            bvr = P.tile([1, C], f32, tag="bvr")
            nc.sync.dma_start(out=bvr, in_=bv_d)
            Gm = P.tile([128, 8], f32, tag="Gm")
            nc.sync.dma_start(out=Gm, in_=g_d)
            Gt = P.tile([8, 128], f32, tag="Gt")
            nc.sync.dma_start(out=Gt, in_=gt_d)
            ones1 = P.tile([1, 128], f32, tag="ones1")
            nc.vector.memset(ones1, 1.0)
            # fp32 x (residual only) — issued last, needed late
            xf_sb = []
            for kc in range(NCH):
                t = P.tile([128, NPOS], f32, tag=f"xf{kc}")
                nc.sync.dma_start(out=t, in_=xm_d[kc * 128:(kc + 1) * 128, :])
                xf_sb.append(t)

            def _reps():
                for _ in range(reps):
                    _body(nc, xb_sb, xo_tiles, xf_sb, wt, gsc, gbi, bqc, bkc,
                          boc, bvr, Gm, Gt, ones1, out_d, ST, QK, VT, ES, AO,
                          RR, OS, PSP, PSS, PSA, row_tile, es_dt)

            if loop_iters is None:
                _reps()
            else:
                with tc.For_i(0, loop_iters, 1):
                    _reps()

    nc.compile()
    return nc


def _body(nc, xb_sb, xo_tiles, xf_sb, wt, gsc, gbi, bqc, bkc, boc, bvr,
          Gm, Gt, ones1, out_d, ST, QK, VT, ES, AO, RR, OS, PSP, PSS, PSA,
          row_tile, es_dt):

    # ================= GroupNorm statistics =================
    # Per-channel mean/E[x^2] over the full batch = own half + other half.
    mv = ST.tile([128, 2 * NCH], f32, tag="mv")  # cols 2k,2k+1 = {mean, E[x^2]}
    statst = []
    for kc in range(NCH):
        stats = ST.tile([128, 8, 6], f32, tag=f"bn{kc}", name=f"bn{kc}")
        xr = xb_sb[kc].rearrange("p (s f) -> p s f", f=512)
        for s in range(4):
            nc.vector.bn_stats(out=stats[:, s, :], in_=xr[:, s, :])
        for hh in range(2):
            xor = xo_tiles[2 * kc + hh].rearrange("p (s f) -> p s f", f=512)
            for s in range(2):
                nc.vector.bn_stats(out=stats[:, 4 + 2 * hh + s, :],
                                   in_=xor[:, s, :])
        statst.append(stats)
    for kc in range(NCH):
        nc.vector.bn_aggr(out=mv[:, 2 * kc:2 * kc + 2], in_=statst[kc])
    # odd cols := var + mean^2 = E[x^2]
    mvr = mv.rearrange("p (k two) -> p k two", two=2)
    msq = ST.tile([128, NCH], f32, tag="msq")
    nc.vector.tensor_tensor(out=msq, in0=mvr[:, :, 0], in1=mvr[:, :, 0],
                            op=OP.mult)
    nc.vector.tensor_tensor(out=mvr[:, :, 1], in0=mvr[:, :, 1], in1=msq,
                            op=OP.add)

    # group sums: one matmul -> [8 local groups, (mean,e) x 4 chunks]
    ps_g = PSS.tile([8, 2 * NCH], f32, tag="pscore", name="ps_g")
    nc.tensor.matmul(ps_g, lhsT=Gm, rhs=mv, start=True, stop=True)
    # mr: cols 0:4 = mu_g, cols 4:8 = rstd_g  (per chunk)
    mr = ST.tile([8, 2 * NCH], f32, tag="mr")
    psr = ps_g.rearrange("p (k two) -> p k two", two=2)
    nc.vector.tensor_scalar_mul(out=mr[:, 0:NCH], in0=psr[:, :, 0],
                                scalar1=1.0 / 16.0)
    nc.vector.tensor_scalar_mul(out=mr[:, NCH:2 * NCH], in0=psr[:, :, 1],
                                scalar1=1.0 / 16.0)
    # var = E[x^2] - mu^2 ; rstd = exp(-0.5*ln(var + eps))
    msq8 = ST.tile([8, NCH], f32, tag="msq8")
    nc.vector.tensor_tensor(out=msq8, in0=mr[:, 0:NCH], in1=mr[:, 0:NCH],
                            op=OP.mult)
    nc.vector.tensor_tensor(out=mr[:, NCH:2 * NCH], in0=mr[:, NCH:2 * NCH],
                            in1=msq8, op=OP.subtract)
    eps8 = ST.tile([8, 1], f32, tag="eps8")
    nc.vector.memset(eps8, EPS)
    nc.scalar.activation(out=mr[:, NCH:2 * NCH], in_=mr[:, NCH:2 * NCH],
                         func=FT.Ln, bias=eps8, scale=1.0)
    nc.scalar.activation(out=mr[:, NCH:2 * NCH], in_=mr[:, NCH:2 * NCH],
                         func=FT.Exp, scale=-0.5)

    # broadcast group stats back to channels (one matmul); A/B per channel
    ps_bc = PSS.tile([128, 2 * NCH], f32, tag="pscore", name="ps_bc")
    nc.tensor.matmul(ps_bc, lhsT=Gt, rhs=mr, start=True, stop=True)
    Acol = ST.tile([128, NCH], f32, tag="Acol")
    Bcol = ST.tile([128, NCH], f32, tag="Bcol")
    nc.vector.tensor_tensor(out=Acol, in0=ps_bc[:, NCH:2 * NCH], in1=gsc,
                            op=OP.mult)
    tb = ST.tile([128, NCH], f32, tag="tb")
    nc.vector.tensor_tensor(out=tb, in0=ps_bc[:, 0:NCH], in1=Acol, op=OP.mult)
    nc.vector.tensor_tensor(out=Bcol, in0=gbi, in1=tb, op=OP.subtract)

    # xn = A*x + B, in place over the bf16 x tiles
    for kc in range(NCH):
        nc.vector.tensor_scalar(out=xb_sb[kc], in0=xb_sb[kc],
                                scalar1=Acol[:, kc:kc + 1],
                                scalar2=Bcol[:, kc:kc + 1],
                                op0=OP.mult, op1=OP.add)
    xn = xb_sb

    # v bias broadcast to all partitions: bvb[p, o] = bv[o]
    ps_bb = PSS.tile([128, C], f32, tag="pscore", name="ps_bb")
    nc.tensor.matmul(ps_bb, lhsT=ones1, rhs=bvr, start=True, stop=True)
    bvb = ST.tile([128, C], f32, tag="bvb")
    nc.vector.tensor_copy(out=bvb, in_=ps_bb)

    # ================= emitters =================
    qk_tiles = []
    for w in range(NWIN):
        q_sb = [QK.tile([128, N], bf16, tag=f"q{kc}", name=f"q{kc}w{w}")
                for kc in range(NCH)]
        k_sb = [QK.tile([128, N], bf16, tag=f"k{kc}", name=f"k{kc}w{w}")
                for kc in range(NCH)]
        qk_tiles.append((q_sb, k_sb))
    vt_tiles = [[VT.tile([128, 1024], es_dt, tag=f"vt{mc}", name=f"vt{mc}w{w}")
                 for mc in range(8)] for w in range(NWIN)]

    def qk_group_emitters(w):
        """16 emitters, oc-major: for each oc, q then k, pc 0/1."""
        q_sb, k_sb = qk_tiles[w]
        base = w * N
        ems = []
        for oc in range(NCH):
            for dst, wf, bcol in ((q_sb, wt["q"], bqc), (k_sb, wt["k"], bkc)):
                for pc in range(2):
                    def em(dst=dst, wf=wf, bcol=bcol, oc=oc, pc=pc):
                        ps = PSP.tile([128, 512], f32, tag="pp", name="ps_qk")
                        for kc in range(NCH):
                            nc.tensor.matmul(
                                ps,
                                lhsT=wf[kc][:, oc * 128:(oc + 1) * 128],
                                rhs=xn[kc][:, base + pc * 512:base + (pc + 1) * 512],
                                start=(kc == 0), stop=(kc == NCH - 1))
                        nc.vector.tensor_scalar(
                            out=dst[oc][:, pc * 512:(pc + 1) * 512], in0=ps,
                            scalar1=bcol[:, oc:oc + 1], scalar2=None,
                            op0=OP.add)
                    ems.append(em)
        return ems

    def vt_emitters(w):
        """8 emitters; vt[mc] head blocks are [v_h(64) | ones(64)]."""
        base = w * N
        ems = []
        for mc in range(8):
            def em(mc=mc, w=w):
                t = vt_tiles[w][mc]
                ps = PSP.tile([128, 512], f32, tag="pp", name="ps_v")
                for kc in range(NCH):
                    nc.tensor.matmul(
                        ps,
                        lhsT=xn[kc][:, base + mc * 128:base + (mc + 1) * 128],
                        rhs=wt["v"][kc],
                        start=(kc == 0), stop=(kc == NCH - 1))
                ap8 = lambda a, off, step: bass.AP(
                    tensor=a.tensor, offset=a.offset + off,
                    ap=[a.ap[0], [step, 8], [1, 64]])
                nc.vector.tensor_tensor(out=ap8(t, 0, 128), in0=ap8(ps, 0, 64),
                                        in1=ap8(bvb, 0, 64), op=OP.add)
                nc.vector.memset(ap8(t, 64, 128), 1.0)
            ems.append(em)
        return ems

    def wo_emitters(w, ao_sb):
        base = w * N
        ems = []
        for oc in range(NCH):
            for nh in range(2):
                def em(oc=oc, nh=nh, base=base, ao_sb=ao_sb):
                    ps_y = PSP.tile([128, 512], f32, tag="pp", name="ps_y")
                    for kc in range(NCH):
                        nc.tensor.matmul(
                            ps_y,
                            lhsT=wt["o"][kc][:, oc * 128:(oc + 1) * 128],
                            rhs=ao_sb[kc][:, nh * 512:(nh + 1) * 512],
                            start=(kc == 0), stop=(kc == NCH - 1))
                    o_t = OS.tile([128, 512], f32, tag="osb", name="o_t")
                    nc.vector.scalar_tensor_tensor(
                        out=o_t, in0=ps_y, scalar=boc[:, oc:oc + 1],
                        in1=xf_sb[oc][:, base + nh * 512:base + (nh + 1) * 512],
                        op0=OP.add, op1=OP.add)
                    nc.sync.dma_start(
                        out=out_d[oc * 128:(oc + 1) * 128,
                                  base + nh * 512:base + (nh + 1) * 512],
                        in_=o_t)
                ems.append(em)
        return ems

    # ================= head-pair machinery =================
    fill = deque()

    def pop_fill(kk):
        for _ in range(kk):
            if fill:
                fill.popleft()()

    def scores_pair(w, j):
        """Scores + exp for heads 2j/2j+1 of window w (row-tiled pair);
        fills interleaved into the exp-gated stretches of the mc loop."""
        q_sb, k_sb = qk_tiles[w]
        esA, esB = [], []
        for mc in range(8):
            psA = PSS.tile([128, N], f32, tag="pscore", name="ps_sA")
            psB = PSS.tile([128, N], f32, tag="pscore", name="ps_sB")
            for nh in range(2):
                nc.tensor.matmul(
                    psA[:, nh * 512:(nh + 1) * 512],
                    lhsT=k_sb[j][0:64, mc * 128:(mc + 1) * 128],
                    rhs=q_sb[j][0:64, nh * 512:(nh + 1) * 512],
                    start=True, stop=True,
                    tile_position=(0, 0) if row_tile else None)
                nc.tensor.matmul(
                    psB[:, nh * 512:(nh + 1) * 512],
                    lhsT=k_sb[j][64:128, mc * 128:(mc + 1) * 128],
                    rhs=q_sb[j][64:128, nh * 512:(nh + 1) * 512],
                    start=True, stop=True,
                    tile_position=(64, 0) if row_tile else None)
            etA = ES.tile([128, N], es_dt, tag=f"esA{mc}", name=f"esA{mc}")
            etB = ES.tile([128, N], es_dt, tag=f"esB{mc}", name=f"esB{mc}")
            nc.scalar.activation(out=etA, in_=psA, func=FT.Exp, scale=SCALE)
            nc.scalar.activation(out=etB, in_=psB, func=FT.Exp, scale=SCALE)
            esA.append(etA)
            esB.append(etB)
            if mc in (1, 3, 5):
                pop_fill(1)
        return esA, esB

    def attn_v(w, h, es_t, ao_sb):
        """attn@V in two 512-halves (one PSUM bank each, double-buffered)
        so the normalize of one half overlaps the accumulation of the
        next. Rowsums sit at psum partitions 64:128 -> shift down (1-input
        copies may cross partitions; reciprocal_approx_fast may not)."""
        ck, po = h // 2, (h % 2) * 64
        for nh in range(2):
            ps_av = PSA.tile([128, 512], f32, tag="pav", name="ps_av")
            for mc in range(8):
                nc.tensor.matmul(ps_av,
                                 lhsT=vt_tiles[w][mc][:, h * 128:(h + 1) * 128],
                                 rhs=es_t[mc][:, nh * 512:(nh + 1) * 512],
                                 start=(mc == 0), stop=(mc == 7))
            rr_t = RR.tile([64, 512], f32, tag=f"rraw{nh}", name="rr_t")
            nc.vector.tensor_copy(out=rr_t, in_=ps_av[64:128, :])
            nc.vector.reciprocal_approx_fast(out=rr_t, in_=rr_t)
            nc.vector.tensor_tensor(
                out=ao_sb[ck][po:po + 64, nh * 512:(nh + 1) * 512],
                in0=ps_av[0:64, :], in1=rr_t, op=OP.mult)

    # ================= schedule =================
    projs = {w: qk_group_emitters(w) for w in range(NWIN)}
    vts = {w: vt_emitters(w) for w in range(NWIN)}
    wo_pending = []

    for w in range(NWIN):
        ao_sb = [AO.tile([128, N], bf16, tag=f"ao{kc}", name=f"ao{kc}w{w}")
                 for kc in range(NCH)]
        if w == 0:
            # lead-in: oc0 q/k so pair 0 can start; the rest feeds the fill
            # queue in dependency order (oc1 before pair 1, all vt before the
            # first attn_v, oc2/oc3 before pairs 2/3, then window-1 work).
            for em in projs[0][0:4]:
                em()
            fill.extend(projs[0][4:8])
            fill.extend(vts[0])
            fill.extend(projs[0][8:])
            fill.extend(projs[1])
            fill.extend(vts[1])
        prev = None
        for j in range(NHEADS // 2):
            if w == 1 and j == 2:
                fill.extend(wo_pending)
                wo_pending = []
            cur = scores_pair(w, j)
            if w == 0 and j == 0:
                pop_fill(9)      # finish oc1 + all of vt under pair-0's exp
            if prev is not None:
                if w == 1 and j == 1:
                    # w1's vt/projections must be emitted before its first AV
                    while fill:
                        fill.popleft()()
                pj = j - 1
                attn_v(w, 2 * pj, prev[0], ao_sb)
                pop_fill(2)
                attn_v(w, 2 * pj + 1, prev[1], ao_sb)
                pop_fill(1)
            prev = cur
        attn_v(w, NHEADS - 2, prev[0], ao_sb)
        pop_fill(2)
        attn_v(w, NHEADS - 1, prev[1], ao_sb)
        pop_fill(2)
        if w == 0:
            wo_pending = wo_emitters(0, ao_sb)
        else:
            for em in wo_emitters(1, ao_sb):
                em()


# ---------------- host-side marshalling ----------------

def _rasterize(xb_half):
    """[C, 32, 64] -> [C, 2048] in (window, row, col) raster order."""
    return np.ascontiguousarray(
        xb_half.reshape(C, WS, 2, WS).transpose(0, 2, 1, 3).reshape(C, NPOS))


def _unrasterize(y):
    """[C, 2048] -> [C, 32, 64]."""
    return y.reshape(C, 2, WS, WS).transpose(0, 2, 1, 3).reshape(C, WS, W)


_NC_CACHE = {}


def _get_nc(reps=1):
    if reps not in _NC_CACHE:
        _NC_CACHE[reps] = build_kernel(reps)
    return _NC_CACHE[reps]


def make_in_maps(x, norm_scale, norm_bias, wq, bq, wk, bk, wv, bv, wo, bo):
    bfnp = mybir.dt.np(bf16)
    x = np.asarray(x, dtype=np.float32)
    xb = x.astype(bfnp)
    cols = lambda v: np.ascontiguousarray(
        np.asarray(v, np.float32).reshape(NCH, 128).T)
    G = np.zeros((128, 8), np.float32)
    for p in range(128):
        G[p, p // 16] = 1.0
    wT = lambda w: np.ascontiguousarray(np.asarray(w, np.float32).T.astype(bfnp))
    shared = {
        "wqT": wT(wq), "wkT": wT(wk), "wvT": wT(wv), "woT": wT(wo),
        "gscale": cols(norm_scale), "gbias": cols(norm_bias),
        "bq": cols(bq), "bk": cols(bk), "bo": cols(bo),
        "bv": np.asarray(bv, np.float32).reshape(1, C).copy(),
        "G": G, "Gt": np.ascontiguousarray(G.T),
    }
    in_maps = []
    for c in range(NCORES):
        b, hi = c // 2, c % 2
        in_maps.append({
            "xb": _rasterize(xb[b, :, hi * WS:(hi + 1) * WS, :]),
            "xob": _rasterize(xb[b, :, (1 - hi) * WS:(1 - hi + 1) * WS, :]),
            "xm": _rasterize(x[b, :, hi * WS:(hi + 1) * WS, :]),
            **shared,
        })
    return in_maps


def kernel(**inputs):
    from concourse.bass_utils import run_bass_kernel_spmd
    nc = _get_nc(1)
    in_maps = make_in_maps(**inputs)
    res = run_bass_kernel_spmd(nc, in_maps, list(range(NCORES)))
    out = np.empty((B, C, H, W), np.float32)
    for c in range(NCORES):
        b, hi = c // 2, c % 2
        out[b, :, hi * WS:(hi + 1) * WS, :] = _unrasterize(res.results[c]["out"])
    return out
